# revision 1
# baseline (speedup 1.0000x reference)
"""GsLmkEncoder Trainium2 kernel.

out[n, b*68+k] = enc_b(n,k) * exp(-0.5 * wq(n,k)),   b in 0..4
  enc_0 = dz = (x_n - l_k) . rz
  enc_1 = sin(dz), enc_2 = cos(dz), enc_3 = sin(2 dz), enc_4 = cos(2 dz)
  wq = (x_n - l_k)^T cov_k (x_n - l_k)

Key reformulation: with s_n = x_n . rz and t_k = l_k . rz, dz = s_n - t_k, so
sin/cos(dz) and sin/cos(2 dz) expand by angle addition into products of
per-point trig (sin s, cos s, sin 2s, cos 2s) and per-landmark trig. wq and dz
are quadratic/linear in x. Everything therefore factors as F[n, 14] @ G[14, 6*68]
with F = [x0^2,x1^2,x2^2,x0x1,x0x2,x1x2,x0,x1,x2,1,sinS,cosS,sin2S,cos2S] and G
host-precomputed from the tiny per-landmark params. On-device per tile of 128
points: one matmul -> PSUM [128, 408] = [wq | dz | sin | cos | sin2 | cos2],
one ACT exp for w = exp(-0.5 wq), one DVE multiply (w broadcast across the 5
blocks), contiguous DMA out. ACT trig runs once in a bulk prologue over all
per-point s values, so the ACT table set switches exactly once (trig -> exp).
"""

import sys
import numpy as np

for _p in ("/opt/trn_rl_repo", "/root/.axon_site/_ro/pypackages"):
    if _p not in sys.path:
        sys.path.insert(0, _p)

import concourse.bass as bass
import concourse.bacc as bacc
import concourse.tile as tile
from concourse import mybir
from concourse.masks import make_identity
from concourse.bass_utils import run_bass_kernel_spmd

# Wire the NTFF profile hook (the agent image's antenv lacks axon_hooks);
# without it trace=True silently degrades to no profiling.
try:
    import antenv.axon_hooks  # noqa: F401
except ImportError:
    try:
        import types as _types

        sys.path.insert(0, "/root/.axon_site")
        from trn_agent_boot.trn_boot import _ntff_profile_via_ctypes

        _hook = _ntff_profile_via_ctypes("/opt/axon/libaxon_pjrt.so")
        _m = _types.ModuleType("antenv.axon_hooks")
        _m.get_axon_ntff_profile_hook = lambda: _hook
        _m.set_axon_ntff_profile_hook = lambda h: None
        sys.modules["antenv.axon_hooks"] = _m
    except Exception:
        pass

F32 = mybir.dt.float32
F32R = mybir.dt.float32r
AF = mybir.ActivationFunctionType
OP = mybir.AluOpType

N = 200000
L = 68
OUT_DIM = 5 * L  # 340
NCOLS = 6 * L    # 408: [wq | dz | s1 | c1 | s2 | c2]
K = 14           # features
NCORES = 8
NTILES = 196                 # tiles of 128 points per shard
NPAD = NTILES * 128          # 25088 per shard
TPF = 3                      # tiles per F-group (transpose granularity)
TPG = 3                      # tiles per psum/out group
KS = 32                      # feature partition stride (PE base-partition rule)
HALF_PI = float(np.pi / 2)
TWO_PI = float(np.float32(2 * np.pi))
PI_F = float(np.float32(np.pi))
INV_2PI = float(np.float32(1.0 / (2 * np.pi)))
INV_PI = float(np.float32(1.0 / np.pi))
MAGIC = 12582912.0  # 1.5 * 2**23: add+sub rounds f32 to nearest int
# clamp bounds keeping func(scale*in+bias) strictly inside [-pi, pi]
B1 = 3.141590
C1 = (-B1, B1)
C1C = (-(B1 + HALF_PI), float(np.float32(B1 - HALF_PI)))
C2 = (-B1 / 2, B1 / 2)
C2C = (-(B1 + HALF_PI) / 2, float(np.float32((B1 - HALF_PI) / 2)))


def _bcast_block(ap, nrep, block):
    """Insert a stride-0 dim of size nrep before the last dim (size block)."""
    new = ap.copy()
    pat = [list(d) for d in new.ap]
    assert pat[-1][1] == block, (pat, block)
    pat.insert(len(pat) - 1, [0, nrep])
    return bass.AP(ap.tensor, ap.offset, pat)


def build_nc(mm_f32r=True, ntiles=NTILES):
    npad = ntiles * 128
    nc = bacc.Bacc("TRN2", target_bir_lowering=False, debug=False, num_devices=NCORES)
    x_d = nc.dram_tensor("x", [npad, 3], F32, kind="ExternalInput")
    g_d = nc.dram_tensor("g", [K, NCOLS], F32, kind="ExternalInput")
    if mm_f32r:
        gt_d = nc.dram_tensor("gt", [4, 4 * L], F32R, kind="ExternalInput")
    rz_d = nc.dram_tensor("rzb", [128, 3], F32, kind="ExternalInput")
    out_d = nc.dram_tensor("out", [npad, OUT_DIM], F32, kind="ExternalOutput")

    # group sizes (tiles per F-group)
    groups = [TPF] * (ntiles // TPF)
    if ntiles % TPF:
        groups.append(ntiles % TPF)

    with tile.TileContext(nc) as tc:
        with (
            tc.tile_pool(name="const", bufs=1) as constp,
            tc.tile_pool(name="fpool", bufs=2) as fpool,
            tc.tile_pool(name="ftpool", bufs=2) as ftpool,
            tc.tile_pool(name="wpool", bufs=3) as wpool,
            tc.tile_pool(name="opool", bufs=4) as opool,
            tc.tile_pool(name="mmps", bufs=2, space="PSUM") as mmpsp,
            tc.tile_pool(name="ftps", bufs=1, space="PSUM") as ftpsp,
        ):
            # ---- persistent tiles ----
            x_sb = constp.tile([128, ntiles, 3], F32)       # grouped point layout
            s_all = constp.tile([128, ntiles], F32)
            ang = constp.tile([128, ntiles, 4], F32)
            scr = constp.tile([128, ntiles], F32)
            trig = constp.tile([128, ntiles, 4], F32)
            g_sb = constp.tile([64 + K, NCOLS], F32)
            if mm_f32r:
                gt_sb = constp.tile([64 + 4, 4 * L], F32R)
            rz_sb = constp.tile([128, 3], F32)
            ident = constp.tile([128, 128], F32)

            for _b in range(3):
                nc.sync.dma_start(g_sb[_b * KS : _b * KS + K, :], g_d[:])
                if mm_f32r:
                    nc.sync.dma_start(gt_sb[_b * KS : _b * KS + 4, :], gt_d[:])
            nc.sync.dma_start(rz_sb[:], rz_d[:])
            make_identity(nc, ident[:])
            bias_hpi = constp.tile([128, 1], F32)
            nc.gpsimd.memset(bias_hpi[:], HALF_PI)
            f_bufs = []
            f2_bufs = []
            for i in range(2):
                fb = fpool.tile([128, TPF * KS], F32, tag=f"F{i}", name=f"F{i}")
                f_bufs.append(fb)
                if mm_f32r:
                    fb2 = fpool.tile(
                        [128, TPF * KS], F32, tag=f"F2{i}", name=f"F2{i}"
                    )
                    f2_bufs.append(fb2)
            for fb in f_bufs + f2_bufs:
                nc.gpsimd.memset(fb[:], 1.0)  # col 9 stays the const-1 feature

            # x load: partition p holds points p*ntiles .. p*ntiles+ntiles-1
            # (one contiguous 2.3KB descriptor per partition)
            nc.sync.dma_start(
                x_sb[:], x_d[:].rearrange("(p m) c -> p m c", p=128)
            )

            # ---- prologue: s = x . rz, then bulk trig ----
            # absorb the x/rz DMA waits on DVE first: TensorScalarPtr
            # encodings only have one sync-wait slot
            nc.vector.tensor_tensor(
                scr[:, 0:3], x_sb[:, 0, :], rz_sb[:, 0:3], OP.mult
            )
            nc.vector.tensor_scalar(
                s_all[:], x_sb[:, :, 0], rz_sb[:, 0:1], None, OP.mult
            )
            nc.vector.scalar_tensor_tensor(
                s_all[:], x_sb[:, :, 1], rz_sb[:, 1:2], s_all[:], OP.mult, OP.add
            )
            nc.vector.scalar_tensor_tensor(
                s_all[:], x_sb[:, :, 2], rz_sb[:, 2:3], s_all[:], OP.mult, OP.add
            )
            # range-reduce the four angle families into [-pi, pi] after
            # the activation's own scale/bias is applied
            fams = [
                (INV_2PI, 0.0, -TWO_PI, C1),     # sin(s)
                (INV_2PI, 0.25, -TWO_PI, C1C),   # sin(s + pi/2)
                (INV_PI, 0.0, -PI_F, C2),        # sin(2s)
                (INV_PI, 0.25, -PI_F, C2C),      # sin(2s + pi/2)
            ]
            for ci, (inv, delta, mul, (lo, hi)) in enumerate(fams):
                # n = round(s*inv + delta) via the 1.5*2^23 magic constant;
                # delta must be added before the magic (ULP there is 1.0)
                nc.vector.tensor_scalar(
                    scr[:], s_all[:], inv, delta, OP.mult, OP.add
                )
                nc.vector.tensor_scalar(
                    scr[:], scr[:], MAGIC, MAGIC, OP.add, OP.subtract
                )
                nc.vector.scalar_tensor_tensor(
                    scr[:], scr[:], mul, s_all[:], OP.mult, OP.add
                )
                nc.vector.tensor_scalar(
                    ang[:, :, ci], scr[:], hi, lo, OP.min, OP.max
                )
            nc.scalar.activation(trig[:, :, 0], ang[:, :, 0], AF.Sin)
            nc.scalar.activation(trig[:, :, 1], ang[:, :, 1], AF.Sin, bias=bias_hpi[:])
            nc.scalar.activation(trig[:, :, 2], ang[:, :, 2], AF.Sin, scale=2.0)
            nc.scalar.activation(
                trig[:, :, 3], ang[:, :, 3], AF.Sin, bias=bias_hpi[:], scale=2.0
            )

            # ---- main loop ----
            col = 0
            gbase = 0
            for gi, tpf in enumerate(groups):
                ncols_f = tpf * KS
                f_t = f_bufs[gi % 2]
                f3 = f_t[:, 0:ncols_f].rearrange("p (t k) -> p t k", k=KS)
                xg = x_sb[:, col : col + tpf, :]
                nc.vector.tensor_tensor(f3[:, :, 0:3], xg, xg, OP.mult)
                nc.vector.tensor_tensor(
                    f3[:, :, 3:4], xg[:, :, 0:1], xg[:, :, 1:2], OP.mult
                )
                nc.vector.tensor_tensor(
                    f3[:, :, 4:5], xg[:, :, 0:1], xg[:, :, 2:3], OP.mult
                )
                nc.vector.tensor_tensor(
                    f3[:, :, 5:6], xg[:, :, 1:2], xg[:, :, 2:3], OP.mult
                )
                nc.vector.tensor_copy(f3[:, :, 6:9], xg)
                if mm_f32r:
                    f2_t = f2_bufs[gi % 2]
                    f23 = f2_t[:, 0:ncols_f].rearrange("p (t k) -> p t k", k=KS)
                    nc.vector.tensor_copy(
                        f23[:, :, 0:4], trig[:, col : col + tpf, :]
                    )
                else:
                    nc.vector.tensor_copy(
                        f3[:, :, 10:14], trig[:, col : col + tpf, :]
                    )

                ft_ps = ftpsp.tile([128, 128], F32, tag="FT")
                nc.tensor.matmul(
                    ft_ps[0:ncols_f, 0:128],
                    f_t[:, 0:ncols_f],
                    ident[:],
                    is_transpose=True,
                )
                ft_sb = ftpool.tile([128, 128], F32, tag="FTS")
                nc.scalar.copy(ft_sb[0:ncols_f, :], ft_ps[0:ncols_f, :])
                if mm_f32r:
                    ft2_ps = ftpsp.tile([128, 128], F32, tag="FT2", name="ft2_ps")
                    nc.tensor.matmul(
                        ft2_ps[0:ncols_f, 0:128],
                        f2_t[:, 0:ncols_f],
                        ident[:],
                        is_transpose=True,
                    )
                    ft2_sb = ftpool.tile([128, 128], F32R, tag="FT2S", name="ft2_sb")
                    nc.scalar.copy(ft2_sb[0:ncols_f, :], ft2_ps[0:ncols_f, :])

                out_rows = out_d[:].rearrange("(p m) c -> p (m c)", p=128)[
                    :, col * OUT_DIM : (col + tpf) * OUT_DIM
                ]

                ogs = [TPG] * (tpf // TPG)
                if tpf % TPG:
                    ogs.append(tpf % TPG)
                j0 = 0
                for tpg in ogs:
                    psum = mmpsp.tile([128, TPG, 512], F32, tag="P")
                    for jj in range(tpg):
                        j = j0 + jj
                        if mm_f32r:
                            nc.tensor.matmul(
                                psum[:, jj, 0 : 2 * L],
                                ft_sb[j * KS : j * KS + 10, 0:128],
                                g_sb[j * KS : j * KS + 10, 0 : 2 * L],
                                start=True,
                                stop=True,
                            )
                            nc.tensor.matmul(
                                psum[:, jj, 2 * L : NCOLS],
                                ft2_sb[j * KS : j * KS + 4, 0:128],
                                gt_sb[j * KS : j * KS + 4, :],
                                start=True,
                                stop=True,
                            )
                        else:
                            nc.tensor.matmul(
                                psum[:, jj, 0:NCOLS],
                                ft_sb[j * KS : j * KS + K, 0:128],
                                g_sb[j * KS : j * KS + K, :],
                                start=True,
                                stop=True,
                            )
                    w_t = wpool.tile([128, TPG, L], F32, tag="W")
                    nc.scalar.activation(
                        w_t[:, 0:tpg, :], psum[:, 0:tpg, 0:L], AF.Exp, scale=-0.5
                    )
                    o_t = opool.tile([128, TPG * OUT_DIM], F32, tag="O")
                    enc = psum[:, 0:tpg, L:NCOLS].rearrange(
                        "p t (b l) -> p t b l", l=L
                    )
                    o4 = o_t[:, 0 : tpg * OUT_DIM].rearrange(
                        "p (t b l) -> p t b l", b=5, l=L
                    )
                    wb = _bcast_block(w_t[:, 0:tpg, :], 5, L)
                    nc.vector.tensor_tensor(o4, enc, wb, OP.mult)
                    nc.sync.dma_start(
                        out_rows[:, j0 * OUT_DIM : (j0 + tpg) * OUT_DIM],
                        o_t[:, 0 : tpg * OUT_DIM],
                    )
                    j0 += tpg
                col += tpf
                gbase += 128 * tpf
    nc.compile()
    return nc


def host_params(l, r, scaling, rotation):
    """G [14, 408] float32 + rz broadcast, mirroring reference math."""
    l = l.astype(np.float64)
    r = r.astype(np.float64)
    scaling = scaling.astype(np.float64)
    rotation = rotation.astype(np.float64)

    rz = r[:3, 2]
    qn = rotation / np.maximum(
        np.linalg.norm(rotation, axis=1, keepdims=True), 1e-12
    )
    w, x, y, z = qn[:, 0], qn[:, 1], qn[:, 2], qn[:, 3]
    R = np.empty((L, 3, 3), np.float64)
    R[:, 0, 0] = 1 - 2 * (y * y + z * z)
    R[:, 0, 1] = 2 * (x * y - w * z)
    R[:, 0, 2] = 2 * (x * z + w * y)
    R[:, 1, 0] = 2 * (x * y + w * z)
    R[:, 1, 1] = 1 - 2 * (x * x + z * z)
    R[:, 1, 2] = 2 * (y * z - w * x)
    R[:, 2, 0] = 2 * (x * z - w * y)
    R[:, 2, 1] = 2 * (y * z + w * x)
    R[:, 2, 2] = 1 - 2 * (x * x + y * y)
    M = R / scaling[:, None, :]
    cov = np.einsum("lij,lkj->lik", M, M)       # [L,3,3]

    b = np.einsum("lij,lj->li", cov, l)         # cov_k @ l_k
    c = np.einsum("li,li->l", l, b)             # l^T cov l
    t = l @ rz
    G = np.zeros((K, NCOLS), np.float64)
    # wq block
    G[0, 0:L] = cov[:, 0, 0]
    G[1, 0:L] = cov[:, 1, 1]
    G[2, 0:L] = cov[:, 2, 2]
    G[3, 0:L] = 2 * cov[:, 0, 1]
    G[4, 0:L] = 2 * cov[:, 0, 2]
    G[5, 0:L] = 2 * cov[:, 1, 2]
    G[6:9, 0:L] = -2 * b.T
    G[9, 0:L] = c
    # dz block
    G[6:9, L : 2 * L] = rz[:, None] * np.ones((1, L))
    G[9, L : 2 * L] = -t
    # trig blocks: sin(s-t) = sinS cosT - cosS sinT ; cos(s-t) = cosS cosT + sinS sinT
    c1, s1 = np.cos(t), np.sin(t)
    c2, s2 = np.cos(2 * t), np.sin(2 * t)
    G[10, 2 * L : 3 * L] = c1
    G[11, 2 * L : 3 * L] = -s1
    G[10, 3 * L : 4 * L] = s1
    G[11, 3 * L : 4 * L] = c1
    G[12, 4 * L : 5 * L] = c2
    G[13, 4 * L : 5 * L] = -s2
    G[12, 5 * L : 6 * L] = s2
    G[13, 5 * L : 6 * L] = c2
    return G.astype(np.float32), np.broadcast_to(
        rz.astype(np.float32), (128, 3)
    ).copy()


_NC_CACHE = {}


def _get_nc(mm_f32r=True):
    key = bool(mm_f32r)
    if key not in _NC_CACHE:
        _NC_CACHE[key] = build_nc(mm_f32r=key)
    return _NC_CACHE[key]


def run(inputs, mm_f32r=True, trace=False):
    x = inputs["x"]
    G, rzb = host_params(
        inputs["l"], inputs["r"], inputs["scaling"], inputs["rotation"]
    )
    xpad = np.zeros((NCORES * NPAD, 3), np.float32)
    xpad[:N] = x
    shards = xpad.reshape(NCORES, NPAD, 3)
    in_maps = []
    for i in range(NCORES):
        m = {"x": np.ascontiguousarray(shards[i]), "g": G, "rzb": rzb}
        if mm_f32r:
            m["gt"] = np.ascontiguousarray(G[10:14, 2 * L :])
        in_maps.append(m)
    nc = _get_nc(mm_f32r)
    res = run_bass_kernel_spmd(nc, in_maps, list(range(NCORES)), trace=trace)
    out = np.concatenate([r["out"] for r in res.results], axis=0)[:N]
    return out, res


def kernel(**inputs):
    out, _ = run(inputs)
    return out



# revision 8
# speedup vs baseline: 1.5540x; 1.5540x over previous
"""GsLmkEncoder Trainium2 kernel.

out[n, b*68+k] = enc_b(n,k) * exp(-0.5 * wq(n,k)),   b in 0..4
  enc_0 = dz = (x_n - l_k) . rz
  enc_1 = sin(dz), enc_2 = cos(dz), enc_3 = sin(2 dz), enc_4 = cos(2 dz)
  wq = (x_n - l_k)^T cov_k (x_n - l_k)

Reformulation: with s_n = x_n . rz and t_k = l_k . rz, dz = s_n - t_k, so
sin/cos(dz) and sin/cos(2 dz) expand by angle addition into products of
per-point trig and per-landmark trig; wq and dz are quadratic/linear in x.
Everything factors as ONE bf16 matmul F[n, 32] @ G[32, 408] per 128-point
tile, where the wq part survives catastrophic cancellation (terms ~1/sigma^2
~ 400 cancel to O(1)) through an error-compensated split: bf16 products are
exact in the f32 PSUM accumulator, so splitting both the quadratic features
and the G entries into bf16 hi/lo parts (F*G ~ Fh*Gh + Fl*Gh + Fh*Gl)
recovers ~f32 accuracy at full bf16 PE speed. The f32 residual of the
per-landmark constant c = l^T cov l has no feature row left (32-row budget);
its low part cl folds into the output as exp(-0.5*cl_k), a bounded
per-landmark factor scaled into G's five enc blocks on the host.

F rows (feature stride 32 so each tile's F^T is 32-aligned for PE row
groups 0/32/64): [qh(6) | ql(6) | qh(6) | xh(3) | xl(3) | xh(3) | 1 |
sinS cosS sin2S cos2S], where q = [x0^2,x1^2,x2^2,x0x1,x1x2,x2x0],
qh = bf16(q) (the f32->bf16 tensor_copy performs the split on-device so it
is bitwise consistent with what the PE sees), ql = q - qh.

Device schedule per shard (25344 points = 66 groups of 3 tiles of 128):
 - bulk prologue: s = x . rz, range-reduce, 4 ACT sins; F assembled by ~10
   whole-shard strided DVE ops into F_all [128, 198, 32] bf16.
 - per 3-tile group: one bf16 PE transpose [128, 96] -> F^T at row groups
   0/32/64; 3 bf16 matmuls [K=32 -> 408 cols] -> PSUM [128, 3, 408+];
   one batched ACT exp -> w; weighted multiply split between DVE (blocks
   dz/sin/cos) and Pool (blocks sin2/cos2) writing fp16; contiguous DMA out.
 - fp16 output halves HBM write traffic; host converts back to f32.
"""

import sys
import numpy as np

for _p in ("/opt/trn_rl_repo", "/root/.axon_site/_ro/pypackages"):
    if _p not in sys.path:
        sys.path.insert(0, _p)

import concourse.bass as bass
import concourse.bacc as bacc
import concourse.tile as tile
from concourse import mybir
from concourse.masks import make_identity
from concourse.bass_utils import run_bass_kernel_spmd

# Wire the NTFF profile hook (the agent image's antenv lacks axon_hooks);
# without it trace=True silently degrades to no profiling.
try:
    import antenv.axon_hooks  # noqa: F401
except ImportError:
    try:
        import types as _types

        sys.path.insert(0, "/root/.axon_site")
        from trn_agent_boot.trn_boot import _ntff_profile_via_ctypes

        _hook = _ntff_profile_via_ctypes("/opt/axon/libaxon_pjrt.so")
        _m = _types.ModuleType("antenv.axon_hooks")
        _m.get_axon_ntff_profile_hook = lambda: _hook
        _m.set_axon_ntff_profile_hook = lambda h: None
        sys.modules["antenv.axon_hooks"] = _m
    except Exception:
        pass

F32 = mybir.dt.float32
F16 = mybir.dt.float16
BF16 = mybir.dt.bfloat16
AF = mybir.ActivationFunctionType
OP = mybir.AluOpType

N = 200000
L = 68
OUT_DIM = 5 * L  # 340
NCOLS = 6 * L    # 408: [wq | dz | s1 | c1 | s2 | c2]
K = 32           # feature rows (full 32-row group)
NCORES = 8
NTILES = 198                 # tiles of 128 points per shard (3 | 198)
NPAD = NTILES * 128          # 25344 per shard
GRP = 3                      # tiles per transpose / psum / out group
NGRP = NTILES // GRP         # 66
KS = 32                      # feature partition stride (PE base-partition rule)
HALF_PI = float(np.pi / 2)
TWO_PI = float(np.float32(2 * np.pi))
PI_F = float(np.float32(np.pi))
INV_2PI = float(np.float32(1.0 / (2 * np.pi)))
INV_PI = float(np.float32(1.0 / np.pi))
MAGIC = 12582912.0  # 1.5 * 2**23: add+sub rounds f32 to nearest int
# clamp bounds keeping func(scale*in+bias) strictly inside [-pi, pi]
B1 = 3.141590
C1 = (-B1, B1)
C1C = (-(B1 + HALF_PI), float(np.float32(B1 - HALF_PI)))
C2 = (-B1 / 2, B1 / 2)
C2C = (-(B1 + HALF_PI) / 2, float(np.float32((B1 - HALF_PI) / 2)))

POOL_MULT = False  # Pool cannot read PSUM (BIR verifier)


def _bcast_block(ap, nrep, block):
    """Insert a stride-0 dim of size nrep before the last dim (size block)."""
    new = ap.copy()
    pat = [list(d) for d in new.ap]
    assert pat[-1][1] == block, (pat, block)
    pat.insert(len(pat) - 1, [0, nrep])
    return bass.AP(ap.tensor, ap.offset, pat)


def bf16_split(a):
    """Round-to-nearest-even bf16 high part (as f64) and residual."""
    a32 = np.asarray(a, np.float32)
    u = a32.view(np.uint32)
    hi = ((u + 0x7FFF + ((u >> 16) & 1)) & 0xFFFF0000).astype(np.uint32)
    h = hi.view(np.float32).astype(np.float64)
    return h, np.asarray(a, np.float64) - h


def build_nc(ntiles=NTILES):
    npad = ntiles * 128
    ngrp = ntiles // GRP
    assert ngrp * GRP == ntiles
    nc = bacc.Bacc("TRN2", target_bir_lowering=False, debug=False, num_devices=NCORES)
    x_d = nc.dram_tensor("x", [npad, 3], F32, kind="ExternalInput")
    g_d = nc.dram_tensor("g", [128, NCOLS], F32, kind="ExternalInput")
    rz_d = nc.dram_tensor("rzb", [128, 3], F32, kind="ExternalInput")
    out_d = nc.dram_tensor("out", [npad, OUT_DIM], BF16, kind="ExternalOutput")

    with tile.TileContext(nc) as tc:
        with (
            tc.tile_pool(name="const", bufs=1) as constp,
            tc.tile_pool(name="wpool", bufs=4) as wpool,
            tc.tile_pool(name="opool", bufs=3) as opool,
            tc.tile_pool(name="mmps", bufs=2, space="PSUM") as mmpsp,
            tc.tile_pool(name="ftps", bufs=2, space="PSUM") as ftpsp,
        ):
            # ---- persistent tiles ----
            x_sb = constp.tile([128, ntiles, 3], F32)       # grouped point layout
            xb_sb = constp.tile([128, ntiles, 3], F32)      # rotated [x1,x2,x0]
            s_all = constp.tile([128, ntiles], F32)
            ang = constp.tile([128, ntiles, 4], F32)
            scr = constp.tile([128, ntiles], F32)
            trig = constp.tile([128, ntiles, 4], F32)
            q32 = constp.tile([128, ntiles, 6], F32)
            qh32 = constp.tile([128, ntiles, 6], F32)
            xh32 = constp.tile([128, ntiles, 3], F32)
            f_all = constp.tile([128, ntiles, KS], BF16)
            ft_all = constp.tile([128, ngrp, 128], BF16)
            g32_sb = constp.tile([128, NCOLS], F32)
            g_sb = constp.tile([128, NCOLS], BF16)
            rz_sb = constp.tile([128, 3], F32)
            ident = constp.tile([128, 128], F32)
            ident_bf = constp.tile([128, 128], BF16)

            nc.sync.dma_start(g32_sb[:], g_d[:])
            nc.sync.dma_start(rz_sb[:], rz_d[:])
            make_identity(nc, ident[:])
            nc.gpsimd.tensor_copy(ident_bf[:], ident[:])
            nc.gpsimd.tensor_copy(g_sb[:], g32_sb[:])
            bias_hpi = constp.tile([128, 1], F32)
            nc.gpsimd.memset(bias_hpi[:], HALF_PI)
            nc.gpsimd.memset(f_all[:, :, 27:28], 1.0)  # const feature
            ncf = GRP * KS  # 96 transposed columns per group

            # x load: partition p holds points p*ntiles .. p*ntiles+ntiles-1
            nc.sync.dma_start(
                x_sb[:], x_d[:].rearrange("(p m) c -> p m c", p=128)
            )

            # ---- prologue: s = x . rz, then bulk trig ----
            # absorb the x/rz DMA waits on DVE first: TensorScalarPtr
            # encodings only have one sync-wait slot
            nc.vector.tensor_tensor(
                scr[:, 0:3], x_sb[:, 0, :], rz_sb[:, 0:3], OP.mult
            )
            nc.vector.tensor_scalar(
                s_all[:], x_sb[:, :, 0], rz_sb[:, 0:1], None, OP.mult
            )
            nc.vector.scalar_tensor_tensor(
                s_all[:], x_sb[:, :, 1], rz_sb[:, 1:2], s_all[:], OP.mult, OP.add
            )
            nc.vector.scalar_tensor_tensor(
                s_all[:], x_sb[:, :, 2], rz_sb[:, 2:3], s_all[:], OP.mult, OP.add
            )
            # range-reduce the four angle families into [-pi, pi] after
            # the activation's own scale/bias is applied
            fams = [
                (INV_2PI, 0.0, -TWO_PI, C1),     # sin(s)
                (INV_2PI, 0.25, -TWO_PI, C1C),   # sin(s + pi/2)
                (INV_PI, 0.0, -PI_F, C2),        # sin(2s)
                (INV_PI, 0.25, -PI_F, C2C),      # sin(2s + pi/2)
            ]
            for ci, (inv, delta, mul, (lo, hi)) in enumerate(fams):
                # n = round(s*inv + delta) via the 1.5*2^23 magic constant;
                # delta must be added before the magic (ULP there is 1.0)
                nc.vector.tensor_scalar(
                    scr[:], s_all[:], inv, delta, OP.mult, OP.add
                )
                nc.vector.tensor_scalar(
                    scr[:], scr[:], MAGIC, MAGIC, OP.add, OP.subtract
                )
                nc.vector.scalar_tensor_tensor(
                    scr[:], scr[:], mul, s_all[:], OP.mult, OP.add
                )
                nc.vector.tensor_scalar(
                    ang[:, :, ci], scr[:], hi, lo, OP.min, OP.max
                )
            nc.scalar.activation(trig[:, :, 0], ang[:, :, 0], AF.Sin)
            nc.scalar.activation(trig[:, :, 1], ang[:, :, 1], AF.Sin, bias=bias_hpi[:])
            nc.scalar.activation(trig[:, :, 2], ang[:, :, 2], AF.Sin, scale=2.0)
            nc.scalar.activation(
                trig[:, :, 3], ang[:, :, 3], AF.Sin, bias=bias_hpi[:], scale=2.0
            )

            # ---- bulk F assembly (whole-shard strided DVE ops) ----
            nc.vector.tensor_copy(xb_sb[:, :, 0:2], x_sb[:, :, 1:3])
            nc.vector.tensor_copy(xb_sb[:, :, 2:3], x_sb[:, :, 0:1])
            nc.vector.tensor_tensor(q32[:, :, 0:3], x_sb[:], x_sb[:], OP.mult)
            nc.vector.tensor_tensor(q32[:, :, 3:6], x_sb[:], xb_sb[:], OP.mult)
            # qh = bf16(q) happens in the f32->bf16 copy; ql = q - qh
            nc.vector.tensor_copy(f_all[:, :, 0:6], q32[:])
            nc.vector.tensor_copy(qh32[:], f_all[:, :, 0:6])
            nc.vector.tensor_tensor(
                f_all[:, :, 6:12], q32[:], qh32[:], OP.subtract
            )
            nc.vector.tensor_copy(f_all[:, :, 12:18], f_all[:, :, 0:6])
            nc.vector.tensor_copy(f_all[:, :, 18:21], x_sb[:])
            nc.vector.tensor_copy(xh32[:], f_all[:, :, 18:21])
            nc.vector.tensor_tensor(
                f_all[:, :, 21:24], x_sb[:], xh32[:], OP.subtract
            )
            nc.vector.tensor_copy(f_all[:, :, 24:27], f_all[:, :, 18:21])
            nc.vector.tensor_copy(f_all[:, :, 28:32], trig[:])

            # ---- main loop: per 3-tile group ----
            for g in range(ngrp):
                ft_ps = ftpsp.tile([128, 128], BF16, tag="FT")
                nc.tensor.matmul(
                    ft_ps[0:ncf, 0:128],
                    f_all[:, g * GRP : (g + 1) * GRP, :],
                    ident_bf[:],
                    is_transpose=True,
                )
                nc.scalar.copy(ft_all[0:ncf, g, :], ft_ps[0:ncf, :])

                out_rows = out_d[:].rearrange("(p m) c -> p (m c)", p=128)[
                    :, g * GRP * OUT_DIM : (g + 1) * GRP * OUT_DIM
                ]
                o_t = opool.tile([128, GRP, 5, L], BF16, tag="O")
                psum = mmpsp.tile([128, GRP, 512], F32, tag="P")
                for j in range(GRP):
                    m = j * KS
                    nc.tensor.matmul(
                        psum[:, j, 0:NCOLS],
                        ft_all[m : m + K, g, :],
                        g_sb[m : m + K, :],
                        start=True,
                        stop=True,
                    )
                w_t = wpool.tile([128, GRP, L], F32, tag="W")
                nc.scalar.activation(
                    w_t[:], psum[:, :, 0:L], AF.Exp, scale=-0.5
                )
                enc3 = psum[:, :, L : 4 * L].rearrange("p t (b l) -> p t b l", l=L)
                enc2 = psum[:, :, 4 * L : NCOLS].rearrange(
                    "p t (b l) -> p t b l", l=L
                )
                wb3 = _bcast_block(w_t[:], 3, L)
                wb2 = _bcast_block(w_t[:], 2, L)
                nc.vector.tensor_tensor(o_t[:, :, 0:3, :], enc3, wb3, OP.mult)
                if POOL_MULT:
                    nc.gpsimd.tensor_tensor(
                        o_t[:, :, 3:5, :], enc2, wb2, OP.mult
                    )
                else:
                    nc.vector.tensor_tensor(
                        o_t[:, :, 3:5, :], enc2, wb2, OP.mult
                    )
                nc.sync.dma_start(
                    out_rows[:], o_t[:].rearrange("p t b l -> p (t b l)")
                )
    nc.compile()
    return nc


def host_params(l, r, scaling, rotation):
    """G [128, 408] f32 (bf16-representable, replicated at 3 row-group
    bases) + rz broadcast. Feature rows per 32-block:
      0-5   qh  * Gh(quad)      6-11  ql * Gh(quad)   12-17 qh * Gl(quad)
      18-20 xh  * bh | rz*eta   21-23 xl * bh | rz*eta
      24-26 xh  * bl            27    1  * ch | -t*eta
      28-31 trig * (per-landmark trig * eta)
    where eta_k = exp(-0.5 * cl_k) folds the bf16 residual of c into the
    five enc blocks."""
    l = l.astype(np.float64)
    r = r.astype(np.float64)
    scaling = scaling.astype(np.float64)
    rotation = rotation.astype(np.float64)

    rz = r[:3, 2]
    qn = rotation / np.maximum(
        np.linalg.norm(rotation, axis=1, keepdims=True), 1e-12
    )
    w, x, y, z = qn[:, 0], qn[:, 1], qn[:, 2], qn[:, 3]
    R = np.empty((L, 3, 3), np.float64)
    R[:, 0, 0] = 1 - 2 * (y * y + z * z)
    R[:, 0, 1] = 2 * (x * y - w * z)
    R[:, 0, 2] = 2 * (x * z + w * y)
    R[:, 1, 0] = 2 * (x * y + w * z)
    R[:, 1, 1] = 1 - 2 * (x * x + z * z)
    R[:, 1, 2] = 2 * (y * z - w * x)
    R[:, 2, 0] = 2 * (x * z - w * y)
    R[:, 2, 1] = 2 * (y * z + w * x)
    R[:, 2, 2] = 1 - 2 * (x * x + y * y)
    M = R / scaling[:, None, :]
    cov = np.einsum("lij,lkj->lik", M, M)       # [L,3,3]

    b = np.einsum("lij,lj->li", cov, l)         # cov_k @ l_k
    c = np.einsum("li,li->l", l, b)             # l^T cov l
    t = l @ rz

    # quad rows matching features [x0^2, x1^2, x2^2, x0x1, x1x2, x2x0]
    Gq = np.stack(
        [
            cov[:, 0, 0],
            cov[:, 1, 1],
            cov[:, 2, 2],
            2 * cov[:, 0, 1],
            2 * cov[:, 1, 2],
            2 * cov[:, 0, 2],
        ]
    )                                            # [6, L]
    Gq_h, Gq_l = bf16_split(Gq)
    b2 = -2.0 * b.T                              # [3, L]
    b2_h, b2_l = bf16_split(b2)
    c_h, c_l = bf16_split(c)
    eta = np.exp(-0.5 * c_l)                     # bounded: |c_l| <= |c|*2^-8

    c1, s1 = np.cos(t), np.sin(t)
    c2, s2 = np.cos(2 * t), np.sin(2 * t)

    G = np.zeros((K, NCOLS), np.float64)
    # wq block
    G[0:6, 0:L] = Gq_h
    G[6:12, 0:L] = Gq_h
    G[12:18, 0:L] = Gq_l
    G[18:21, 0:L] = b2_h
    G[21:24, 0:L] = b2_h
    G[24:27, 0:L] = b2_l
    G[27, 0:L] = c_h
    # dz block (xh + xl recovers full-precision x)
    G[18:21, L : 2 * L] = rz[:, None] * eta[None, :]
    G[21:24, L : 2 * L] = rz[:, None] * eta[None, :]
    G[27, L : 2 * L] = -t * eta
    # trig blocks: sin(s-t) = sinS cosT - cosS sinT ; cos(s-t) = ...
    G[28, 2 * L : 3 * L] = c1 * eta
    G[29, 2 * L : 3 * L] = -s1 * eta
    G[28, 3 * L : 4 * L] = s1 * eta
    G[29, 3 * L : 4 * L] = c1 * eta
    G[30, 4 * L : 5 * L] = c2 * eta
    G[31, 4 * L : 5 * L] = -s2 * eta
    G[30, 5 * L : 6 * L] = s2 * eta
    G[31, 5 * L : 6 * L] = c2 * eta
    # pre-round everything to bf16-representable f32 so the host splits
    # are exactly what the PE multiplies
    Gbf, _ = bf16_split(G)
    Grep = np.zeros((128, NCOLS), np.float32)
    for m in range(GRP):
        Grep[m * KS : m * KS + K, :] = Gbf.astype(np.float32)
    return Grep, np.broadcast_to(
        rz.astype(np.float32), (128, 3)
    ).copy()


_NC_CACHE = {}


def _get_nc():
    if "nc" not in _NC_CACHE:
        _NC_CACHE["nc"] = build_nc()
    return _NC_CACHE["nc"]


def run(inputs, mm_f32r=True, trace=False):
    x = inputs["x"]
    G, rzb = host_params(
        inputs["l"], inputs["r"], inputs["scaling"], inputs["rotation"]
    )
    xpad = np.zeros((NCORES * NPAD, 3), np.float32)
    xpad[:N] = x
    shards = xpad.reshape(NCORES, NPAD, 3)
    in_maps = [
        {"x": np.ascontiguousarray(shards[i]), "g": G, "rzb": rzb}
        for i in range(NCORES)
    ]
    nc = _get_nc()
    res = run_bass_kernel_spmd(nc, in_maps, list(range(NCORES)), trace=trace)
    out = np.concatenate([r["out"] for r in res.results], axis=0)[:N]
    return out.astype(np.float32), res


def kernel(**inputs):
    out, _ = run(inputs)
    return out
